# revision 8
# baseline (speedup 1.0000x reference)
"""DenseCapsLayer Trainium2 kernel, v2.

Math (per (n, a); A=32 in-caps, B=32 out-caps, P=4, hw=256, 3 routing iters):
  votes v[h,b] = W[a,b] @ M[h]  -- never materialized.
  Mbar[b] = sum_h softmax_h(L)[h,b] * M[h]
  Z[b]    = G[a,b] @ Mbar[b],  G = W^T W   (fp32, host-precomputed)
  n2      = <Mbar, Z>  (clamped >= 0), f = squash factor
  U[b]    = f * Z[b];   L[h,b] = M[h] . Ubar[b]  (Ubar = cumulative U)
  out     = f * (W @ Mbar) at iter 2.

v2 structural changes vs v1:
  - iter-0 state U0 computed in host prep (softmax at t=0 is uniform, so
    Mbar0 = mean_h M is a linear map of the input). Device starts at L1.
  - single-precision x per path: bf16 for Mb matmuls, fp16 for L matmuls.
  - softmax denominator via a ones-column baked into the Mb moving operand.
  - U^T via one PE-array transpose per batch-half; L matmuls use 32-row PE
    tiles at 32-aligned tile_positions with host-baked zeros selecting the
    j-parity.
  - per-batch-half chains software-pipelined against the other half's
    PE phases; input DMAs need-ordered on one queue, aux tensors packed
    into a single f32 DMA with bitcast views.

Sharding: data-parallel over batch, core c handles n in {2c, 2c+1} (nl=2).
Partitions: (aL, b) = aL*32 + b with a = 4j + aL, j = 0..7.
"""

import numpy as np
import ml_dtypes

import concourse.bass as bass
import concourse.bacc as bacc
import concourse.mybir as mybir
import concourse.tile as tile
from concourse.bass_utils import run_bass_kernel_spmd

F32 = mybir.dt.float32
F16 = mybir.dt.float16
BF16 = mybir.dt.bfloat16

A, B, P, ITERS = 32, 32, 4, 3
PS = P * P                      # 16
BATCH, OH, OW = 16, 16, 16
HW = OH * OW                    # 256
NCORES = 8
NL = BATCH // NCORES            # 2
J = 8                           # j blocks (a = 4j + aL)
EPS = 1e-8

AF = mybir.ActivationFunctionType
ALU = mybir.AluOpType
AX = mybir.AxisListType

import os as _os
_STOP = _os.environ.get("K_STOP", "")


# ---------------------------------------------------------------- device code
def _emit(tc, mt32d, u0t32d, auxd, xmbd, o32):
    nc = tc.nc

    dbg_view = o32.rearrange("n a b k -> (n a b k)") \
                  .rearrange("(p f) -> p f", f=256)

    def dump(src):
        nc.sync.dma_start(out=dbg_view, in_=src)

    with (
        tc.tile_pool(name="inp", bufs=1) as inp,
        tc.tile_pool(name="state", bufs=1) as state,
        tc.tile_pool(name="work", bufs=2) as work,
        tc.tile_pool(name="small", bufs=2) as small,
        tc.tile_pool(name="lps", bufs=3, space="PSUM") as lps_pool,
        tc.tile_pool(name="mbps", bufs=1, space="PSUM") as mbps_pool,
        tc.tile_pool(name="utps", bufs=1, space="PSUM") as utps_pool,
    ):
        # ---------------- inputs, DMA'd in need-order on the sync queue
        # aux packs [ident(64 f32-cols), wga(512), u0ub(64), wws(512)]
        u0t32 = inp.tile([128, 2 * 128], F16, tag="u0t32")
        nc.sync.dma_start(out=u0t32[:], in_=u0t32d)
        mt32 = inp.tile([128, 4096], F16, tag="mt32")
        xmb = inp.tile([128, 2 * 2 * 520], BF16, tag="xmb")
        aux = inp.tile([128, 1216], F32, tag="aux")
        for j2 in range(4):
            nc.sync.dma_start(
                out=mt32[j2 * 32:(j2 + 1) * 32, 0:2048],
                in_=mt32d[j2 * 32:(j2 + 1) * 32, 0:2048])
        nc.sync.dma_start(out=xmb[:, 0:1040], in_=xmbd[:, 0:1040])
        nc.sync.dma_start(out=aux[:], in_=auxd)
        nc.sync.dma_start(out=mt32[:, 2048:4096], in_=mt32d[:, 2048:4096])
        nc.sync.dma_start(out=xmb[:, 1040:2080], in_=xmbd[:, 1040:2080])
        ident = aux[:, 0:64].bitcast(F16)
        wga = aux[:, 64:576]
        u0ub = aux[:, 576:704].bitcast(F16)
        wws = aux[:, 704:1216]
        epsc = inp.tile([128, 1], F32, tag="epsc")
        nc.gpsimd.memset(epsc[:], EPS)

        # preload combined exp+ln activation table set once
        from concourse.hw_specs import get_activation_tables
        _tables = list(get_activation_tables(nc.m.arch).items())
        _set_id = next(i for i, (nm, fns) in enumerate(_tables)
                       if AF.Exp in fns and AF.Ln in fns)
        nc.scalar.add_instruction(mybir.InstLoadActFuncSet(
            name=nc.get_next_instruction_name(),
            ins=[], outs=[], act_func_set_id=_set_id))

        # el[nl]: exp(L) in bf16, cols j*256 + ch*128 + (aL*32 + b)
        el = {}
        for nl in range(NL):
            t_el = state.tile([128, 2048], BF16, tag=f"el{nl}", name="t_el")
            el[nl] = t_el

        def emit_L_mms(nl, rhs_tile, rhs_col0, jps=(0, 1, 2, 3)):
            """L matmuls + exp for one batch-half. rhs rows (j,kq)=j*16+kq,
            cols (aL, b) at rhs_col0."""
            for jp in jps:
                lp = lps_pool.tile([128, 512], F32, tag="lp", name="lp")
                for jo in range(2):
                    j = jp * 2 + jo
                    j2 = j // 2           # == jp
                    for ch in range(2):
                        for aL in range(4):
                            a_col = ((nl * 2 + (j % 2)) * 4 + aL) * 256 \
                                + ch * 128
                            lhsT = mt32[j2 * 32:(j2 + 1) * 32,
                                        a_col:a_col + 128]
                            rhs = rhs_tile[j2 * 32:(j2 + 1) * 32,
                                           rhs_col0 + aL * 32:
                                           rhs_col0 + (aL + 1) * 32]
                            nc.tensor.matmul(
                                lp[:, jo * 256 + ch * 128 + aL * 32:
                                   jo * 256 + ch * 128 + (aL + 1) * 32],
                                lhsT, rhs, start=True, stop=True,
                                tile_position=(j2 * 32, 0))
                nc.scalar.activation(el[nl][:, jp * 512:(jp + 1) * 512],
                                     lp[:], AF.Exp)

        def emit_mb(nl, jhs=(0, 1), mbps=None):
            """Mb matmuls for one batch-half -> 2 psum tiles (jh halves)."""
            if mbps is None:
                mbps = []
            for jh in jhs:
                mp = mbps_pool.tile([128, 260], F32, tag=f"mb{nl}{jh}",
                                    name="mp")
                mbps.append(mp)
                for j4 in range(4):
                    j = jh * 4 + j4
                    for ch in range(2):
                        lhsT = el[nl][:, j * 256 + ch * 128:
                                      j * 256 + (ch + 1) * 128]
                        rhs = xmb[:, nl * 1040 + ch * 520 + j * 65:
                                  nl * 1040 + ch * 520 + (j + 1) * 65]
                        nc.tensor.matmul(mp[:, j4 * 65:(j4 + 1) * 65],
                                         lhsT, rhs,
                                         start=(ch == 0), stop=(ch == 1))
            return mbps

        def emit_recd(nl, mbps):
            recds = []
            for jh in range(2):
                mpv = mbps[jh][:].rearrange("p (j c) -> p j c", c=65)
                rc = small.tile([128, 4], F32, tag=f"recd{nl}{jh}",
                                name="rc")
                nc.vector.reciprocal(rc[:], mpv[:, :, 64])
                recds.append(rc)
            return recds

        def emit_extract(nl, mbps, recds, mbar, use_act=False):
            """Diag extraction + normalize. With use_act (Act idle): 8 Act
            copies to f32 scratch + 2 DVE normalize-mults; else 8 DVE ops."""
            mview = mbar[:].rearrange("p (j kq) -> p j kq", kq=PS)
            if use_act:
                mbu = work.tile([128, 128], F32, tag=f"mbu{nl}",
                                name="mbu")
                muv = mbu[:].rearrange("p (j kq) -> p j kq", kq=PS)
            for jh in range(2):
                mpv = mbps[jh][:].rearrange("p (j c) -> p j c", c=65)
                for aL in range(4):
                    src = mpv[aL * 32:(aL + 1) * 32, :,
                              aL * 16:aL * 16 + 16]
                    if use_act:
                        dst = muv[aL * 32:(aL + 1) * 32,
                                  jh * 4:(jh + 1) * 4]
                        nc.scalar.activation(dst, src, AF.Copy)
                    else:
                        rb = recds[jh][aL * 32:(aL + 1) * 32] \
                            .unsqueeze(2).broadcast_to((32, 4, PS))
                        dst = mview[aL * 32:(aL + 1) * 32,
                                    jh * 4:(jh + 1) * 4]
                        nc.vector.tensor_tensor(dst, src, rb, op=ALU.mult)
            if use_act:
                for jh in range(2):
                    rb = recds[jh][:].unsqueeze(2) \
                        .broadcast_to((128, 4, PS))
                    nc.vector.tensor_tensor(
                        mview[:, jh * 4:(jh + 1) * 4],
                        muv[:, jh * 4:(jh + 1) * 4], rb, op=ALU.mult)

        def emit_squash(nl, mbar, t):
            """Per-nl squash chain. Returns (sv, ff2)."""
            mview = mbar[:].rearrange("p (j kq) -> p j kq", kq=PS)
            if t == 1:
                tz = work.tile([128, 512], F32, tag=f"tz{nl}", name="tz")
                z = state.tile([128, 128], F32, tag=f"z{nl}", name="z")
                tzv = tz[:].rearrange("p (j kp k q) -> p j kp k q",
                                      kp=4, k=4, q=4)
                gv = wga.rearrange("p (j kp k q) -> p j kp k q",
                                   kp=4, k=4, q=4)
                mbv = mview.rearrange("p j (kp q) -> p j kp q", q=4) \
                    .unsqueeze(3).broadcast_to((128, 8, 4, 4, 4))
                nc.vector.tensor_tensor(tzv, gv, mbv, op=ALU.mult)
                nc.vector.tensor_reduce(
                    out=z[:].rearrange("p (j k q) -> p j k q", k=4, q=4),
                    in_=tz[:].rearrange("p (j kp k q) -> p j k q kp",
                                        kp=4, k=4, q=4),
                    op=ALU.add, axis=AX.X)
                sv = z
            else:
                ts = work.tile([128, 512], F32, tag=f"tz{nl}", name="ts")
                s = state.tile([128, 128], F32, tag=f"s{nl}", name="s")
                tsv = ts[:].rearrange("p (j k pp q) -> p j k pp q",
                                      k=4, pp=4, q=4)
                wv = wws.rearrange("p (j k pp q) -> p j k pp q",
                                   k=4, pp=4, q=4)
                mbv = mview.rearrange("p j (k q) -> p j k q", q=4) \
                    .unsqueeze(3).broadcast_to((128, 8, 4, 4, 4))
                nc.vector.tensor_tensor(tsv, wv, mbv, op=ALU.mult)
                nc.vector.tensor_reduce(
                    out=s[:].rearrange("p (j pq) -> p j pq", pq=PS),
                    in_=ts[:].rearrange("p (j k pp q) -> p j pp q k",
                                        k=4, pp=4, q=4),
                    op=ALU.add, axis=AX.X)
                sv = s
            mz = work.tile([128, 128], F32, tag=f"mz{nl}", name="mz")
            n2 = small.tile([128, 8], F32, tag=f"n2{nl}", name="n2")
            meng = nc.gpsimd if (t == 2 and nl == 0) else nc.vector
            meng.tensor_tensor(
                mz[:], (mbar[:] if t == 1 else sv[:]), sv[:], op=ALU.mult)
            nc.vector.tensor_reduce(
                out=n2[:], in_=mz[:].rearrange("p (j kq) -> p j kq",
                                               kq=PS),
                op=ALU.add, axis=AX.X)
            n2c = small.tile([128, 8], F32, tag=f"n2c{nl}", name="n2c")
            nc.vector.tensor_scalar_max(n2c[:], n2[:], 0.0)
            # f = n2/(1+n2) * (n2+eps)^-0.5 via Ln/Exp (v1-proven path);
            # dd/rec/ff overlap the Act round-trip on DVE.
            tln = small.tile([128, 8], F32, tag=f"tln{nl}", name="tln")
            nc.scalar.activation(tln[:], n2c[:], AF.Ln, bias=epsc[:])
            dd = small.tile([128, 8], F32, tag=f"dd{nl}", name="dd")
            nc.vector.tensor_scalar_add(dd[:], n2c[:], 1.0)
            rec = small.tile([128, 8], F32, tag=f"rec{nl}", name="rec")
            nc.vector.reciprocal(rec[:], dd[:])
            rr = small.tile([128, 8], F32, tag=f"rr{nl}", name="rr")
            nc.scalar.activation(rr[:], tln[:], AF.Exp, scale=-0.5)
            ff = small.tile([128, 8], F32, tag=f"ff{nl}", name="ff")
            nc.vector.tensor_mul(ff[:], n2c[:], rec[:])
            ff2 = small.tile([128, 8], F32, tag=f"ff2{nl}", name="ff2")
            nc.vector.tensor_mul(ff2[:], ff[:], rr[:])
            return sv, ff2

        def emit_chain1(nl):
            """t=1 chain for one nl: recd/extract/squash/u16/ub."""
            mbps = mbps_t1[nl]
            recds = emit_recd(nl, mbps)
            mbar = state.tile([128, 128], F16, tag=f"mbar{nl}",
                              name="mbar")
            emit_extract(nl, mbps, recds, mbar)
            z, ff2 = emit_squash(nl, mbar, 1)
            fbc = ff2[:].unsqueeze(2).broadcast_to((128, 8, PS))
            u16 = state.tile([128, 128], F16, tag=f"u16{nl}", name="u16")
            nc.vector.tensor_tensor(
                u16[:].rearrange("p (j kq) -> p j kq", kq=PS),
                z[:].rearrange("p (j kq) -> p j kq", kq=PS),
                fbc, op=ALU.mult)
            ubt = state.tile([128, 128], F16, tag=f"ub{nl}", name="ubt")
            nc.vector.tensor_add(
                ubt[:], u0ub[:, nl * 128:(nl + 1) * 128], u16[:])
            return ubt

        utp_big = utps_pool.tile([128, 256], F16, tag="utp",
                                 name="utp_big")

        def emit_transp(nl, ubt):
            c0 = nl * 128
            nc.tensor.transpose(utp_big[:, c0:c0 + 128], ubt[:], ident)
            uta = state.tile([128, 128], F16, tag=f"uta{nl}", name="uta")
            nc.scalar.activation(uta[:], utp_big[:, c0:c0 + 128], AF.Copy)
            return uta

        def emit_chain2(nl):
            """t=2 chain for one nl: recd/extract/S/out + DMA."""
            mbps = mbps_t2[nl]
            recds = emit_recd(nl, mbps)
            mbar = state.tile([128, 128], F16, tag=f"mbar2{nl}",
                              name="mbar")
            emit_extract(nl, mbps, recds, mbar)
            s, ff2 = emit_squash(nl, mbar, 2)
            fbc = ff2[:].unsqueeze(2).broadcast_to((128, 8, PS))
            outsb = state.tile([128, 128], F32, tag=f"outsb{nl}",
                               name="outsb")
            oeng = nc.gpsimd if nl == 0 else nc.vector
            oeng.tensor_tensor(
                outsb[:].rearrange("p (j kq) -> p j kq", kq=PS),
                s[:].rearrange("p (j kq) -> p j kq", kq=PS),
                fbc, op=ALU.mult)
            src_o = outsb[:].rearrange("p (jj kq) -> p jj kq", kq=PS)
            dst_o = o32[nl].rearrange("(jj aL) b kq -> (aL b) jj kq", jj=J)
            nc.sync.dma_start(out=dst_o, in_=src_o)

        # ================= pipelined schedule
        mbps_t1 = {}
        mbps_t2 = {}

        emit_L_mms(0, u0t32, 0)            # PE: L1-nl0
        mbps_t1[0] = emit_mb(0)            # PE: Mb1-nl0
        emit_L_mms(1, u0t32, 128)          # PE: L1-nl1  (chain1-nl0 overlaps)
        ub0 = emit_chain1(0)
        mbps_t1[1] = emit_mb(1)            # PE: Mb1-nl1
        uta0 = emit_transp(0, ub0)
        ub1 = emit_chain1(1)               # overlaps L2-nl0
        emit_L_mms(0, uta0, 0)             # PE: L2-nl0
        mbps_t2[0] = emit_mb(0)            # PE: Mb2-nl0
        uta1 = emit_transp(1, ub1)
        emit_chain2(0)                     # overlaps L2-nl1 on PE
        emit_L_mms(1, uta1, 0, jps=(0, 1))     # PE: L2-nl1 j0-3
        mbps_t2[1] = emit_mb(1, jhs=(0,))      # PE: Mb2-nl1 jh0
        emit_L_mms(1, uta1, 0, jps=(2, 3))     # PE: L2-nl1 j4-7
        emit_mb(1, jhs=(1,), mbps=mbps_t2[1])  # PE: Mb2-nl1 jh1
        emit_chain2(1)


def _build_kernel():
    nc = bacc.Bacc("TRN2", target_bir_lowering=False, debug=False,
                   num_devices=NCORES)
    mt32d = nc.dram_tensor("mt32", [128, 4096], F16,
                           kind="ExternalInput").ap()
    u0t32d = nc.dram_tensor("u0t32", [128, 256], F16,
                            kind="ExternalInput").ap()
    auxd = nc.dram_tensor("aux", [128, 1216], F32,
                          kind="ExternalInput").ap()
    xmbd = nc.dram_tensor("xmb", [128, 2080], BF16,
                          kind="ExternalInput").ap()
    o32 = nc.dram_tensor("o32", [NL, A, B, PS], F32,
                         kind="ExternalOutput").ap()

    with tile.TileContext(nc) as tc:
        _emit(tc, mt32d, u0t32d, auxd, xmbd, o32)

    nc.compile()
    return nc


# ---------------------------------------------------------------- host side
def _squashf(n2):
    n2c = np.maximum(n2, 0.0)
    return (n2c / (1.0 + n2c)) / np.sqrt(n2c + EPS)


def _host_prep(x, weights):
    xr = np.asarray(x, np.float32).reshape(BATCH, HW, A, PS)
    W = np.asarray(weights, np.float32)
    Gm = np.einsum("abpk,abpl->abkl", W, W)
    Gsw = np.swapaxes(Gm, 2, 3)                    # [a,b,kp,k] = Gm[..,k,kp]
    Wsw = np.swapaxes(W, 2, 3)                     # [a,b,k,pp] = W[..,pp,k]

    # wga[aL*32+b, j*64 + kp*16 + k*4 + q] = Gsw[4j+aL, b, kp, k]
    wga = np.empty((4, B, J, 4, 4, 4), np.float32)
    wws = np.empty((4, B, J, 4, 4, 4), np.float32)
    for j in range(J):
        wga[:, :, j] = Gsw[4 * j:4 * j + 4, :, :, :, None]
        wws[:, :, j] = Wsw[4 * j:4 * j + 4, :, :, :, None]
    wga = np.ascontiguousarray(wga.reshape(128, 512))
    wws = np.ascontiguousarray(wws.reshape(128, 512))
    ident = np.eye(128, dtype=np.float16)

    x16 = xr.astype(np.float16)                    # L-path
    xbf = xr.astype(ml_dtypes.bfloat16)            # Mb-path

    in_maps = []
    for c in range(NCORES):
        xc16 = x16[c * NL:(c + 1) * NL]            # (2, 256, 32, 16)
        xcbf = xbf[c * NL:(c + 1) * NL]
        xcf = xr[c * NL:(c + 1) * NL]

        # host U0 (t=0 state; softmax at t=0 is uniform)
        Mbar0 = xcf.mean(axis=1)                   # (2, A, PS)
        Z0 = np.einsum("abkl,nalq->nabkq", Gm,
                       Mbar0.reshape(NL, A, P, P)).reshape(NL, A, B, PS)
        n2_0 = np.einsum("nak,nabk->nab", Mbar0, Z0)[..., None]
        U0 = (_squashf(n2_0) * Z0).astype(np.float16)   # (2, A, B, PS)

        # u0t32[j*16+kq, nl*128 + aL*32 + b] ; u0ub[aL*32+b, nl*128+(j,kq)]
        u0t32 = np.zeros((128, 256), np.float16)
        u0ub = np.zeros((128, 256), np.float16)
        for nl in range(NL):
            for j in range(J):
                for aL in range(4):
                    blk = U0[nl, 4 * j + aL]       # (B, PS)
                    u0t32[j * 16:(j + 1) * 16,
                          nl * 128 + aL * 32:nl * 128 + (aL + 1) * 32] = \
                        blk.T
                    u0ub[aL * 32:(aL + 1) * 32,
                         nl * 128 + j * 16:nl * 128 + (j + 1) * 16] = blk

        # mt32[j2*32 + par*16 + kq, ((nl*2+par)*4+aL)*256 + ch*128 + h]
        mt32 = np.zeros((128, 4096), np.float16)
        for j in range(J):
            j2, par = j // 2, j % 2
            for nl in range(NL):
                for aL in range(4):
                    for ch in range(2):
                        col = ((nl * 2 + par) * 4 + aL) * 256 + ch * 128
                        mt32[j2 * 32 + par * 16:j2 * 32 + par * 16 + 16,
                             col:col + 128] = \
                            xc16[nl, ch * 128:(ch + 1) * 128,
                                 4 * j + aL, :].T

        # xmb[h, nl*1040 + ch*520 + j*65 + (aL*16+kq | 64)]
        xmb = np.empty((128, 2080), ml_dtypes.bfloat16)
        xv = xmb.reshape(128, NL, 2, J, 65)
        for nl in range(NL):
            for ch in range(2):
                xv[:, nl, ch, :, :64] = (
                    xcbf[nl, ch * 128:(ch + 1) * 128]
                    .reshape(128, J, 64))
                xv[:, nl, ch, :, 64] = 1.0

        aux = np.concatenate([
            ident.view(np.float32), wga, u0ub.view(np.float32), wws,
        ], axis=1)
        in_maps.append({
            "mt32": mt32,
            "u0t32": u0t32,
            "aux": np.ascontiguousarray(aux),
            "xmb": np.ascontiguousarray(xmb),
        })
    return in_maps


_NC_CACHE = {}


def kernel(x, weights):
    if "nc" not in _NC_CACHE:
        _NC_CACHE["nc"] = _build_kernel()
    nc = _NC_CACHE["nc"]
    in_maps = _host_prep(x, weights)
    res = run_bass_kernel_spmd(nc, in_maps, list(range(NCORES)))
    out = np.concatenate([res.results[c]["o32"] for c in range(NCORES)],
                         axis=0)
    return out.astype(np.float32)


# revision 9
# speedup vs baseline: 1.0263x; 1.0263x over previous
"""DenseCapsLayer Trainium2 kernel, v2.

Math (per (n, a); A=32 in-caps, B=32 out-caps, P=4, hw=256, 3 routing iters):
  votes v[h,b] = W[a,b] @ M[h]  -- never materialized.
  Mbar[b] = sum_h softmax_h(L)[h,b] * M[h]
  Z[b]    = G[a,b] @ Mbar[b],  G = W^T W   (fp32, host-precomputed)
  n2      = <Mbar, Z>  (clamped >= 0), f = squash factor
  U[b]    = f * Z[b];   L[h,b] = M[h] . Ubar[b]  (Ubar = cumulative U)
  out     = f * (W @ Mbar) at iter 2.

v2 structural changes vs v1:
  - iter-0 state U0 computed in host prep (softmax at t=0 is uniform, so
    Mbar0 = mean_h M is a linear map of the input). Device starts at L1.
  - single-precision x per path: bf16 for Mb matmuls, fp16 for L matmuls.
  - softmax denominator via a ones-column baked into the Mb moving operand.
  - U^T via one PE-array transpose per batch-half; L matmuls use 32-row PE
    tiles at 32-aligned tile_positions with host-baked zeros selecting the
    j-parity.
  - per-batch-half chains software-pipelined against the other half's
    PE phases; input DMAs need-ordered on one queue, aux tensors packed
    into a single f32 DMA with bitcast views.

Sharding: data-parallel over batch, core c handles n in {2c, 2c+1} (nl=2).
Partitions: (aL, b) = aL*32 + b with a = 4j + aL, j = 0..7.
"""

import numpy as np
import ml_dtypes

import concourse.bass as bass
import concourse.bacc as bacc
import concourse.mybir as mybir
import concourse.tile as tile
from concourse.bass_utils import run_bass_kernel_spmd

F32 = mybir.dt.float32
F16 = mybir.dt.float16
BF16 = mybir.dt.bfloat16

A, B, P, ITERS = 32, 32, 4, 3
PS = P * P                      # 16
BATCH, OH, OW = 16, 16, 16
HW = OH * OW                    # 256
NCORES = 8
NL = BATCH // NCORES            # 2
J = 8                           # j blocks (a = 4j + aL)
EPS = 1e-8

AF = mybir.ActivationFunctionType
ALU = mybir.AluOpType
AX = mybir.AxisListType

import os as _os
_STOP = _os.environ.get("K_STOP", "")


# ---------------------------------------------------------------- device code
def _emit(tc, mt32d, u0t32d, auxd, xmbd, o32):
    nc = tc.nc

    dbg_view = o32.rearrange("n a b k -> (n a b k)") \
                  .rearrange("(p f) -> p f", f=256)

    def dump(src):
        nc.sync.dma_start(out=dbg_view, in_=src)

    with (
        tc.tile_pool(name="inp", bufs=1) as inp,
        tc.tile_pool(name="state", bufs=1) as state,
        tc.tile_pool(name="work", bufs=2) as work,
        tc.tile_pool(name="small", bufs=2) as small,
        tc.tile_pool(name="lps", bufs=3, space="PSUM") as lps_pool,
        tc.tile_pool(name="mbps", bufs=1, space="PSUM") as mbps_pool,
        tc.tile_pool(name="utps", bufs=1, space="PSUM") as utps_pool,
    ):
        # ---------------- inputs, DMA'd in need-order on the sync queue
        # aux packs [ident(64 f32-cols), wga(512), u0ub(64), wws(512)]
        u0t32 = inp.tile([128, 2 * 128], F16, tag="u0t32")
        nc.scalar.dma_start(out=u0t32[:], in_=u0t32d)
        mt32 = inp.tile([128, 4096], F16, tag="mt32")
        xmb = inp.tile([128, 2 * 2 * 520], BF16, tag="xmb")
        aux = inp.tile([128, 1216], F32, tag="aux")
        for j2 in range(2):
            nc.gpsimd.dma_start(out=mt32[j2 * 32:(j2 + 1) * 32, 0:2048],
                                in_=mt32d[j2 * 32:(j2 + 1) * 32, 0:2048])
        for j2 in range(2, 4):
            nc.sync.dma_start(
                out=mt32[j2 * 32:(j2 + 1) * 32, 0:2048],
                in_=mt32d[j2 * 32:(j2 + 1) * 32, 0:2048])
        nc.sync.dma_start(out=xmb[:, 0:1040], in_=xmbd[:, 0:1040])
        nc.sync.dma_start(out=aux[:], in_=auxd)
        nc.sync.dma_start(out=mt32[:, 2048:4096], in_=mt32d[:, 2048:4096])
        nc.sync.dma_start(out=xmb[:, 1040:2080], in_=xmbd[:, 1040:2080])
        ident = aux[:, 0:64].bitcast(F16)
        wga = aux[:, 64:576]
        u0ub = aux[:, 576:704].bitcast(F16)
        wws = aux[:, 704:1216]
        epsc = inp.tile([128, 1], F32, tag="epsc")
        nc.gpsimd.memset(epsc[:], EPS)

        # preload combined exp+ln activation table set once
        from concourse.hw_specs import get_activation_tables
        _tables = list(get_activation_tables(nc.m.arch).items())
        _set_id = next(i for i, (nm, fns) in enumerate(_tables)
                       if AF.Exp in fns and AF.Ln in fns)
        nc.scalar.add_instruction(mybir.InstLoadActFuncSet(
            name=nc.get_next_instruction_name(),
            ins=[], outs=[], act_func_set_id=_set_id))

        # el[nl]: exp(L) in bf16, cols j*256 + ch*128 + (aL*32 + b)
        el = {}
        for nl in range(NL):
            t_el = state.tile([128, 2048], BF16, tag=f"el{nl}", name="t_el")
            el[nl] = t_el

        def emit_L_mms(nl, rhs_tile, rhs_col0, jps=(0, 1, 2, 3)):
            """L matmuls + exp for one batch-half. rhs rows (j,kq)=j*16+kq,
            cols (aL, b) at rhs_col0."""
            for jp in jps:
                lp = lps_pool.tile([128, 512], F32, tag="lp", name="lp")
                for jo in range(2):
                    j = jp * 2 + jo
                    j2 = j // 2           # == jp
                    for ch in range(2):
                        for aL in range(4):
                            a_col = ((nl * 2 + (j % 2)) * 4 + aL) * 256 \
                                + ch * 128
                            lhsT = mt32[j2 * 32:(j2 + 1) * 32,
                                        a_col:a_col + 128]
                            rhs = rhs_tile[j2 * 32:(j2 + 1) * 32,
                                           rhs_col0 + aL * 32:
                                           rhs_col0 + (aL + 1) * 32]
                            nc.tensor.matmul(
                                lp[:, jo * 256 + ch * 128 + aL * 32:
                                   jo * 256 + ch * 128 + (aL + 1) * 32],
                                lhsT, rhs, start=True, stop=True,
                                tile_position=(j2 * 32, 0))
                nc.scalar.activation(el[nl][:, jp * 512:(jp + 1) * 512],
                                     lp[:], AF.Exp)

        def emit_mb(nl, jhs=(0, 1), mbps=None):
            """Mb matmuls for one batch-half -> 2 psum tiles (jh halves)."""
            if mbps is None:
                mbps = []
            for jh in jhs:
                mp = mbps_pool.tile([128, 260], F32, tag=f"mb{nl}{jh}",
                                    name="mp")
                mbps.append(mp)
                for j4 in range(4):
                    j = jh * 4 + j4
                    for ch in range(2):
                        lhsT = el[nl][:, j * 256 + ch * 128:
                                      j * 256 + (ch + 1) * 128]
                        rhs = xmb[:, nl * 1040 + ch * 520 + j * 65:
                                  nl * 1040 + ch * 520 + (j + 1) * 65]
                        nc.tensor.matmul(mp[:, j4 * 65:(j4 + 1) * 65],
                                         lhsT, rhs,
                                         start=(ch == 0), stop=(ch == 1))
            return mbps

        def emit_recd(nl, mbps):
            recds = []
            for jh in range(2):
                mpv = mbps[jh][:].rearrange("p (j c) -> p j c", c=65)
                rc = small.tile([128, 4], F32, tag=f"recd{nl}{jh}",
                                name="rc")
                nc.vector.reciprocal(rc[:], mpv[:, :, 64])
                recds.append(rc)
            return recds

        def emit_extract(nl, mbps, recds, mbar, use_act=False):
            """Diag extraction + normalize. With use_act (Act idle): 8 Act
            copies to f32 scratch + 2 DVE normalize-mults; else 8 DVE ops."""
            mview = mbar[:].rearrange("p (j kq) -> p j kq", kq=PS)
            if use_act:
                mbu = work.tile([128, 128], F32, tag=f"mbu{nl}",
                                name="mbu")
                muv = mbu[:].rearrange("p (j kq) -> p j kq", kq=PS)
            for jh in range(2):
                mpv = mbps[jh][:].rearrange("p (j c) -> p j c", c=65)
                for aL in range(4):
                    src = mpv[aL * 32:(aL + 1) * 32, :,
                              aL * 16:aL * 16 + 16]
                    if use_act:
                        dst = muv[aL * 32:(aL + 1) * 32,
                                  jh * 4:(jh + 1) * 4]
                        nc.scalar.activation(dst, src, AF.Copy)
                    else:
                        rb = recds[jh][aL * 32:(aL + 1) * 32] \
                            .unsqueeze(2).broadcast_to((32, 4, PS))
                        dst = mview[aL * 32:(aL + 1) * 32,
                                    jh * 4:(jh + 1) * 4]
                        nc.vector.tensor_tensor(dst, src, rb, op=ALU.mult)
            if use_act:
                for jh in range(2):
                    rb = recds[jh][:].unsqueeze(2) \
                        .broadcast_to((128, 4, PS))
                    nc.vector.tensor_tensor(
                        mview[:, jh * 4:(jh + 1) * 4],
                        muv[:, jh * 4:(jh + 1) * 4], rb, op=ALU.mult)

        def emit_squash(nl, mbar, t):
            """Per-nl squash chain. Returns (sv, ff2)."""
            mview = mbar[:].rearrange("p (j kq) -> p j kq", kq=PS)
            if t == 1:
                tz = work.tile([128, 512], F32, tag=f"tz{nl}", name="tz")
                z = state.tile([128, 128], F32, tag=f"z{nl}", name="z")
                tzv = tz[:].rearrange("p (j kp k q) -> p j kp k q",
                                      kp=4, k=4, q=4)
                gv = wga.rearrange("p (j kp k q) -> p j kp k q",
                                   kp=4, k=4, q=4)
                mbv = mview.rearrange("p j (kp q) -> p j kp q", q=4) \
                    .unsqueeze(3).broadcast_to((128, 8, 4, 4, 4))
                nc.vector.tensor_tensor(tzv, gv, mbv, op=ALU.mult)
                nc.vector.tensor_reduce(
                    out=z[:].rearrange("p (j k q) -> p j k q", k=4, q=4),
                    in_=tz[:].rearrange("p (j kp k q) -> p j k q kp",
                                        kp=4, k=4, q=4),
                    op=ALU.add, axis=AX.X)
                sv = z
            else:
                ts = work.tile([128, 512], F32, tag=f"tz{nl}", name="ts")
                s = state.tile([128, 128], F32, tag=f"s{nl}", name="s")
                tsv = ts[:].rearrange("p (j k pp q) -> p j k pp q",
                                      k=4, pp=4, q=4)
                wv = wws.rearrange("p (j k pp q) -> p j k pp q",
                                   k=4, pp=4, q=4)
                mbv = mview.rearrange("p j (k q) -> p j k q", q=4) \
                    .unsqueeze(3).broadcast_to((128, 8, 4, 4, 4))
                nc.vector.tensor_tensor(tsv, wv, mbv, op=ALU.mult)
                nc.vector.tensor_reduce(
                    out=s[:].rearrange("p (j pq) -> p j pq", pq=PS),
                    in_=ts[:].rearrange("p (j k pp q) -> p j pp q k",
                                        k=4, pp=4, q=4),
                    op=ALU.add, axis=AX.X)
                sv = s
            mz = work.tile([128, 128], F32, tag=f"mz{nl}", name="mz")
            n2 = small.tile([128, 8], F32, tag=f"n2{nl}", name="n2")
            meng = nc.gpsimd if (t == 2 and nl == 0) else nc.vector
            meng.tensor_tensor(
                mz[:], (mbar[:] if t == 1 else sv[:]), sv[:], op=ALU.mult)
            nc.vector.tensor_reduce(
                out=n2[:], in_=mz[:].rearrange("p (j kq) -> p j kq",
                                               kq=PS),
                op=ALU.add, axis=AX.X)
            n2c = small.tile([128, 8], F32, tag=f"n2c{nl}", name="n2c")
            nc.vector.tensor_scalar_max(n2c[:], n2[:], 0.0)
            # f = n2/(1+n2) * (n2+eps)^-0.5 via Ln/Exp (v1-proven path);
            # dd/rec/ff overlap the Act round-trip on DVE.
            tln = small.tile([128, 8], F32, tag=f"tln{nl}", name="tln")
            nc.scalar.activation(tln[:], n2c[:], AF.Ln, bias=epsc[:])
            dd = small.tile([128, 8], F32, tag=f"dd{nl}", name="dd")
            nc.vector.tensor_scalar_add(dd[:], n2c[:], 1.0)
            rec = small.tile([128, 8], F32, tag=f"rec{nl}", name="rec")
            nc.vector.reciprocal(rec[:], dd[:])
            rr = small.tile([128, 8], F32, tag=f"rr{nl}", name="rr")
            nc.scalar.activation(rr[:], tln[:], AF.Exp, scale=-0.5)
            ff = small.tile([128, 8], F32, tag=f"ff{nl}", name="ff")
            nc.vector.tensor_mul(ff[:], n2c[:], rec[:])
            ff2 = small.tile([128, 8], F32, tag=f"ff2{nl}", name="ff2")
            nc.vector.tensor_mul(ff2[:], ff[:], rr[:])
            return sv, ff2

        def emit_chain1(nl):
            """t=1 chain for one nl: recd/extract/squash/u16/ub."""
            mbps = mbps_t1[nl]
            recds = emit_recd(nl, mbps)
            mbar = state.tile([128, 128], F16, tag=f"mbar{nl}",
                              name="mbar")
            emit_extract(nl, mbps, recds, mbar)
            z, ff2 = emit_squash(nl, mbar, 1)
            fbc = ff2[:].unsqueeze(2).broadcast_to((128, 8, PS))
            u16 = state.tile([128, 128], F16, tag=f"u16{nl}", name="u16")
            nc.vector.tensor_tensor(
                u16[:].rearrange("p (j kq) -> p j kq", kq=PS),
                z[:].rearrange("p (j kq) -> p j kq", kq=PS),
                fbc, op=ALU.mult)
            ubt = state.tile([128, 128], F16, tag=f"ub{nl}", name="ubt")
            nc.vector.tensor_add(
                ubt[:], u0ub[:, nl * 128:(nl + 1) * 128], u16[:])
            return ubt

        utp_big = utps_pool.tile([128, 256], F16, tag="utp",
                                 name="utp_big")

        def emit_transp(nl, ubt):
            c0 = nl * 128
            nc.tensor.transpose(utp_big[:, c0:c0 + 128], ubt[:], ident)
            uta = state.tile([128, 128], F16, tag=f"uta{nl}", name="uta")
            nc.scalar.activation(uta[:], utp_big[:, c0:c0 + 128], AF.Copy)
            return uta

        def emit_chain2(nl):
            """t=2 chain for one nl: recd/extract/S/out + DMA."""
            mbps = mbps_t2[nl]
            recds = emit_recd(nl, mbps)
            mbar = state.tile([128, 128], F16, tag=f"mbar2{nl}",
                              name="mbar")
            emit_extract(nl, mbps, recds, mbar)
            s, ff2 = emit_squash(nl, mbar, 2)
            fbc = ff2[:].unsqueeze(2).broadcast_to((128, 8, PS))
            outsb = state.tile([128, 128], F32, tag=f"outsb{nl}",
                               name="outsb")
            oeng = nc.gpsimd if nl == 0 else nc.vector
            oeng.tensor_tensor(
                outsb[:].rearrange("p (j kq) -> p j kq", kq=PS),
                s[:].rearrange("p (j kq) -> p j kq", kq=PS),
                fbc, op=ALU.mult)
            src_o = outsb[:].rearrange("p (jj kq) -> p jj kq", kq=PS)
            dst_o = o32[nl].rearrange("(jj aL) b kq -> (aL b) jj kq", jj=J)
            nc.sync.dma_start(out=dst_o, in_=src_o)

        # ================= pipelined schedule
        mbps_t1 = {}
        mbps_t2 = {}

        emit_L_mms(0, u0t32, 0)            # PE: L1-nl0
        mbps_t1[0] = emit_mb(0)            # PE: Mb1-nl0
        emit_L_mms(1, u0t32, 128)          # PE: L1-nl1  (chain1-nl0 overlaps)
        ub0 = emit_chain1(0)
        mbps_t1[1] = emit_mb(1)            # PE: Mb1-nl1
        uta0 = emit_transp(0, ub0)
        ub1 = emit_chain1(1)               # overlaps L2-nl0
        emit_L_mms(0, uta0, 0)             # PE: L2-nl0
        mbps_t2[0] = emit_mb(0)            # PE: Mb2-nl0
        uta1 = emit_transp(1, ub1)
        emit_chain2(0)                     # overlaps L2-nl1 on PE
        emit_L_mms(1, uta1, 0, jps=(0, 1))     # PE: L2-nl1 j0-3
        mbps_t2[1] = emit_mb(1, jhs=(0,))      # PE: Mb2-nl1 jh0
        emit_L_mms(1, uta1, 0, jps=(2, 3))     # PE: L2-nl1 j4-7
        emit_mb(1, jhs=(1,), mbps=mbps_t2[1])  # PE: Mb2-nl1 jh1
        emit_chain2(1)


def _build_kernel():
    nc = bacc.Bacc("TRN2", target_bir_lowering=False, debug=False,
                   num_devices=NCORES)
    mt32d = nc.dram_tensor("mt32", [128, 4096], F16,
                           kind="ExternalInput").ap()
    u0t32d = nc.dram_tensor("u0t32", [128, 256], F16,
                            kind="ExternalInput").ap()
    auxd = nc.dram_tensor("aux", [128, 1216], F32,
                          kind="ExternalInput").ap()
    xmbd = nc.dram_tensor("xmb", [128, 2080], BF16,
                          kind="ExternalInput").ap()
    o32 = nc.dram_tensor("o32", [NL, A, B, PS], F32,
                         kind="ExternalOutput").ap()

    with tile.TileContext(nc) as tc:
        _emit(tc, mt32d, u0t32d, auxd, xmbd, o32)

    nc.compile()
    return nc


# ---------------------------------------------------------------- host side
def _squashf(n2):
    n2c = np.maximum(n2, 0.0)
    return (n2c / (1.0 + n2c)) / np.sqrt(n2c + EPS)


def _host_prep(x, weights):
    xr = np.asarray(x, np.float32).reshape(BATCH, HW, A, PS)
    W = np.asarray(weights, np.float32)
    Gm = np.einsum("abpk,abpl->abkl", W, W)
    Gsw = np.swapaxes(Gm, 2, 3)                    # [a,b,kp,k] = Gm[..,k,kp]
    Wsw = np.swapaxes(W, 2, 3)                     # [a,b,k,pp] = W[..,pp,k]

    # wga[aL*32+b, j*64 + kp*16 + k*4 + q] = Gsw[4j+aL, b, kp, k]
    wga = np.empty((4, B, J, 4, 4, 4), np.float32)
    wws = np.empty((4, B, J, 4, 4, 4), np.float32)
    for j in range(J):
        wga[:, :, j] = Gsw[4 * j:4 * j + 4, :, :, :, None]
        wws[:, :, j] = Wsw[4 * j:4 * j + 4, :, :, :, None]
    wga = np.ascontiguousarray(wga.reshape(128, 512))
    wws = np.ascontiguousarray(wws.reshape(128, 512))
    ident = np.eye(128, dtype=np.float16)

    x16 = xr.astype(np.float16)                    # L-path
    xbf = xr.astype(ml_dtypes.bfloat16)            # Mb-path

    in_maps = []
    for c in range(NCORES):
        xc16 = x16[c * NL:(c + 1) * NL]            # (2, 256, 32, 16)
        xcbf = xbf[c * NL:(c + 1) * NL]
        xcf = xr[c * NL:(c + 1) * NL]

        # host U0 (t=0 state; softmax at t=0 is uniform)
        Mbar0 = xcf.mean(axis=1)                   # (2, A, PS)
        Z0 = np.einsum("abkl,nalq->nabkq", Gm,
                       Mbar0.reshape(NL, A, P, P)).reshape(NL, A, B, PS)
        n2_0 = np.einsum("nak,nabk->nab", Mbar0, Z0)[..., None]
        U0 = (_squashf(n2_0) * Z0).astype(np.float16)   # (2, A, B, PS)

        # u0t32[j*16+kq, nl*128 + aL*32 + b] ; u0ub[aL*32+b, nl*128+(j,kq)]
        u0t32 = np.zeros((128, 256), np.float16)
        u0ub = np.zeros((128, 256), np.float16)
        for nl in range(NL):
            for j in range(J):
                for aL in range(4):
                    blk = U0[nl, 4 * j + aL]       # (B, PS)
                    u0t32[j * 16:(j + 1) * 16,
                          nl * 128 + aL * 32:nl * 128 + (aL + 1) * 32] = \
                        blk.T
                    u0ub[aL * 32:(aL + 1) * 32,
                         nl * 128 + j * 16:nl * 128 + (j + 1) * 16] = blk

        # mt32[j2*32 + par*16 + kq, ((nl*2+par)*4+aL)*256 + ch*128 + h]
        mt32 = np.zeros((128, 4096), np.float16)
        for j in range(J):
            j2, par = j // 2, j % 2
            for nl in range(NL):
                for aL in range(4):
                    for ch in range(2):
                        col = ((nl * 2 + par) * 4 + aL) * 256 + ch * 128
                        mt32[j2 * 32 + par * 16:j2 * 32 + par * 16 + 16,
                             col:col + 128] = \
                            xc16[nl, ch * 128:(ch + 1) * 128,
                                 4 * j + aL, :].T

        # xmb[h, nl*1040 + ch*520 + j*65 + (aL*16+kq | 64)]
        xmb = np.empty((128, 2080), ml_dtypes.bfloat16)
        xv = xmb.reshape(128, NL, 2, J, 65)
        for nl in range(NL):
            for ch in range(2):
                xv[:, nl, ch, :, :64] = (
                    xcbf[nl, ch * 128:(ch + 1) * 128]
                    .reshape(128, J, 64))
                xv[:, nl, ch, :, 64] = 1.0

        aux = np.concatenate([
            ident.view(np.float32), wga, u0ub.view(np.float32), wws,
        ], axis=1)
        in_maps.append({
            "mt32": mt32,
            "u0t32": u0t32,
            "aux": np.ascontiguousarray(aux),
            "xmb": np.ascontiguousarray(xmb),
        })
    return in_maps


_NC_CACHE = {}


def kernel(x, weights):
    if "nc" not in _NC_CACHE:
        _NC_CACHE["nc"] = _build_kernel()
    nc = _NC_CACHE["nc"]
    in_maps = _host_prep(x, weights)
    res = run_bass_kernel_spmd(nc, in_maps, list(range(NCORES)))
    out = np.concatenate([res.results[c]["o32"] for c in range(NCORES)],
                         axis=0)
    return out.astype(np.float32)


# revision 10
# speedup vs baseline: 1.0343x; 1.0078x over previous
"""DenseCapsLayer Trainium2 kernel, v2.

Math (per (n, a); A=32 in-caps, B=32 out-caps, P=4, hw=256, 3 routing iters):
  votes v[h,b] = W[a,b] @ M[h]  -- never materialized.
  Mbar[b] = sum_h softmax_h(L)[h,b] * M[h]
  Z[b]    = G[a,b] @ Mbar[b],  G = W^T W   (fp32, host-precomputed)
  n2      = <Mbar, Z>  (clamped >= 0), f = squash factor
  U[b]    = f * Z[b];   L[h,b] = M[h] . Ubar[b]  (Ubar = cumulative U)
  out     = f * (W @ Mbar) at iter 2.

v2 structural changes vs v1:
  - iter-0 state U0 computed in host prep (softmax at t=0 is uniform, so
    Mbar0 = mean_h M is a linear map of the input). Device starts at L1.
  - single-precision x per path: bf16 for Mb matmuls, fp16 for L matmuls.
  - softmax denominator via a ones-column baked into the Mb moving operand.
  - U^T via one PE-array transpose per batch-half; L matmuls use 32-row PE
    tiles at 32-aligned tile_positions with host-baked zeros selecting the
    j-parity.
  - per-batch-half chains software-pipelined against the other half's
    PE phases; input DMAs need-ordered on one queue, aux tensors packed
    into a single f32 DMA with bitcast views.

Sharding: data-parallel over batch, core c handles n in {2c, 2c+1} (nl=2).
Partitions: (aL, b) = aL*32 + b with a = 4j + aL, j = 0..7.
"""

import numpy as np
import ml_dtypes

import concourse.bass as bass
import concourse.bacc as bacc
import concourse.mybir as mybir
import concourse.tile as tile
from concourse.bass_utils import run_bass_kernel_spmd

F32 = mybir.dt.float32
F16 = mybir.dt.float16
BF16 = mybir.dt.bfloat16

A, B, P, ITERS = 32, 32, 4, 3
PS = P * P                      # 16
BATCH, OH, OW = 16, 16, 16
HW = OH * OW                    # 256
NCORES = 8
NL = BATCH // NCORES            # 2
J = 8                           # j blocks (a = 4j + aL)
EPS = 1e-8

AF = mybir.ActivationFunctionType
ALU = mybir.AluOpType
AX = mybir.AxisListType

import os as _os
_STOP = _os.environ.get("K_STOP", "")


# ---------------------------------------------------------------- device code
def _emit(tc, mt32d, u0t32d, auxd, xmbd, o32):
    nc = tc.nc

    dbg_view = o32.rearrange("n a b k -> (n a b k)") \
                  .rearrange("(p f) -> p f", f=256)

    def dump(src):
        nc.sync.dma_start(out=dbg_view, in_=src)

    with (
        tc.tile_pool(name="inp", bufs=1) as inp,
        tc.tile_pool(name="state", bufs=1) as state,
        tc.tile_pool(name="work", bufs=2) as work,
        tc.tile_pool(name="small", bufs=2) as small,
        tc.tile_pool(name="lps", bufs=3, space="PSUM") as lps_pool,
        tc.tile_pool(name="mbps", bufs=1, space="PSUM") as mbps_pool,
        tc.tile_pool(name="utps", bufs=1, space="PSUM") as utps_pool,
    ):
        # ---------------- inputs, DMA'd in need-order on the sync queue
        # aux packs [ident(64 f32-cols), wga(512), u0ub(64), wws(512)]
        u0t32 = inp.tile([128, 2 * 128], F16, tag="u0t32")
        nc.scalar.dma_start(out=u0t32[:], in_=u0t32d)
        mt32 = inp.tile([128, 4096], F16, tag="mt32")
        xmb = inp.tile([128, 2 * 2 * 520], BF16, tag="xmb")
        aux = inp.tile([128, 1216], F32, tag="aux")
        for j2 in range(2):
            nc.gpsimd.dma_start(out=mt32[j2 * 32:(j2 + 1) * 32, 0:2048],
                                in_=mt32d[j2 * 32:(j2 + 1) * 32, 0:2048])
        for j2 in range(2, 4):
            nc.sync.dma_start(
                out=mt32[j2 * 32:(j2 + 1) * 32, 0:2048],
                in_=mt32d[j2 * 32:(j2 + 1) * 32, 0:2048])
        nc.sync.dma_start(out=xmb[:, 0:1040], in_=xmbd[:, 0:1040])
        nc.sync.dma_start(out=aux[:], in_=auxd)
        nc.sync.dma_start(out=mt32[:, 2048:4096], in_=mt32d[:, 2048:4096])
        nc.sync.dma_start(out=xmb[:, 1040:2080], in_=xmbd[:, 1040:2080])
        ident = aux[:, 0:64].bitcast(F16)
        wga = aux[:, 64:576]
        u0ub = aux[:, 576:704].bitcast(F16)
        wws = aux[:, 704:1216]
        epsc = inp.tile([128, 1], F32, tag="epsc")
        nc.gpsimd.memset(epsc[:], EPS)

        # preload combined exp+ln activation table set once
        from concourse.hw_specs import get_activation_tables
        _tables = list(get_activation_tables(nc.m.arch).items())
        _set_id = next(i for i, (nm, fns) in enumerate(_tables)
                       if AF.Exp in fns and AF.Ln in fns)
        nc.scalar.add_instruction(mybir.InstLoadActFuncSet(
            name=nc.get_next_instruction_name(),
            ins=[], outs=[], act_func_set_id=_set_id))

        # el[nl]: exp(L) in bf16, cols j*256 + ch*128 + (aL*32 + b)
        el = {}
        for nl in range(NL):
            t_el = state.tile([128, 2048], BF16, tag=f"el{nl}", name="t_el")
            el[nl] = t_el

        def emit_L_mms(nl, rhs_tile, rhs_col0, jps=(0, 1, 2, 3)):
            """L matmuls + exp for one batch-half. rhs rows (j,kq)=j*16+kq,
            cols (aL, b) at rhs_col0."""
            for jp in jps:
                lp = lps_pool.tile([128, 512], F32, tag="lp", name="lp")
                for jo in range(2):
                    j = jp * 2 + jo
                    j2 = j // 2           # == jp
                    for ch in range(2):
                        for aL in range(4):
                            a_col = ((nl * 2 + (j % 2)) * 4 + aL) * 256 \
                                + ch * 128
                            lhsT = mt32[j2 * 32:(j2 + 1) * 32,
                                        a_col:a_col + 128]
                            rhs = rhs_tile[j2 * 32:(j2 + 1) * 32,
                                           rhs_col0 + aL * 32:
                                           rhs_col0 + (aL + 1) * 32]
                            nc.tensor.matmul(
                                lp[:, jo * 256 + ch * 128 + aL * 32:
                                   jo * 256 + ch * 128 + (aL + 1) * 32],
                                lhsT, rhs, start=True, stop=True,
                                tile_position=(j2 * 32, 0))
                nc.scalar.activation(el[nl][:, jp * 512:(jp + 1) * 512],
                                     lp[:], AF.Exp)

        def emit_mb(nl, jhs=(0, 1), mbps=None):
            """Mb matmuls for one batch-half -> 2 psum tiles (jh halves)."""
            if mbps is None:
                mbps = []
            for jh in jhs:
                mp = mbps_pool.tile([128, 260], F32, tag=f"mb{nl}{jh}",
                                    name="mp")
                mbps.append(mp)
                for j4 in range(4):
                    j = jh * 4 + j4
                    for ch in range(2):
                        lhsT = el[nl][:, j * 256 + ch * 128:
                                      j * 256 + (ch + 1) * 128]
                        rhs = xmb[:, nl * 1040 + ch * 520 + j * 65:
                                  nl * 1040 + ch * 520 + (j + 1) * 65]
                        nc.tensor.matmul(mp[:, j4 * 65:(j4 + 1) * 65],
                                         lhsT, rhs,
                                         start=(ch == 0), stop=(ch == 1))
            return mbps

        def emit_recd(nl, mbps):
            recds = []
            for jh in range(2):
                mpv = mbps[jh][:].rearrange("p (j c) -> p j c", c=65)
                rc = small.tile([128, 4], F32, tag=f"recd{nl}{jh}",
                                name="rc")
                nc.vector.reciprocal(rc[:], mpv[:, :, 64])
                recds.append(rc)
            return recds

        def emit_extract(nl, mbps, recds, mbar, use_act=False):
            """Diag extraction + normalize. With use_act (Act idle): 8 Act
            copies to f32 scratch + 2 DVE normalize-mults; else 8 DVE ops."""
            mview = mbar[:].rearrange("p (j kq) -> p j kq", kq=PS)
            if use_act:
                mbu = work.tile([128, 128], F32, tag=f"mbu{nl}",
                                name="mbu")
                muv = mbu[:].rearrange("p (j kq) -> p j kq", kq=PS)
            for jh in range(2):
                mpv = mbps[jh][:].rearrange("p (j c) -> p j c", c=65)
                for aL in range(4):
                    src = mpv[aL * 32:(aL + 1) * 32, :,
                              aL * 16:aL * 16 + 16]
                    if use_act:
                        dst = muv[aL * 32:(aL + 1) * 32,
                                  jh * 4:(jh + 1) * 4]
                        nc.scalar.activation(dst, src, AF.Copy)
                    else:
                        rb = recds[jh][aL * 32:(aL + 1) * 32] \
                            .unsqueeze(2).broadcast_to((32, 4, PS))
                        dst = mview[aL * 32:(aL + 1) * 32,
                                    jh * 4:(jh + 1) * 4]
                        nc.vector.tensor_tensor(dst, src, rb, op=ALU.mult)
            if use_act:
                for jh in range(2):
                    rb = recds[jh][:].unsqueeze(2) \
                        .broadcast_to((128, 4, PS))
                    nc.vector.tensor_tensor(
                        mview[:, jh * 4:(jh + 1) * 4],
                        muv[:, jh * 4:(jh + 1) * 4], rb, op=ALU.mult)

        def emit_squash(nl, mbar, t):
            """Per-nl squash chain. Returns (sv, ff2)."""
            mview = mbar[:].rearrange("p (j kq) -> p j kq", kq=PS)
            if t == 1:
                tz = work.tile([128, 512], F32, tag=f"tz{nl}", name="tz")
                z = state.tile([128, 128], F32, tag=f"z{nl}", name="z")
                tzv = tz[:].rearrange("p (j kp k q) -> p j kp k q",
                                      kp=4, k=4, q=4)
                gv = wga.rearrange("p (j kp k q) -> p j kp k q",
                                   kp=4, k=4, q=4)
                mbv = mview.rearrange("p j (kp q) -> p j kp q", q=4) \
                    .unsqueeze(3).broadcast_to((128, 8, 4, 4, 4))
                nc.vector.tensor_tensor(tzv, gv, mbv, op=ALU.mult)
                nc.vector.tensor_reduce(
                    out=z[:].rearrange("p (j k q) -> p j k q", k=4, q=4),
                    in_=tz[:].rearrange("p (j kp k q) -> p j k q kp",
                                        kp=4, k=4, q=4),
                    op=ALU.add, axis=AX.X)
                sv = z
            else:
                ts = work.tile([128, 512], F32, tag=f"tz{nl}", name="ts")
                s = state.tile([128, 128], F32, tag=f"s{nl}", name="s")
                tsv = ts[:].rearrange("p (j k pp q) -> p j k pp q",
                                      k=4, pp=4, q=4)
                wv = wws.rearrange("p (j k pp q) -> p j k pp q",
                                   k=4, pp=4, q=4)
                mbv = mview.rearrange("p j (k q) -> p j k q", q=4) \
                    .unsqueeze(3).broadcast_to((128, 8, 4, 4, 4))
                nc.vector.tensor_tensor(tsv, wv, mbv, op=ALU.mult)
                nc.vector.tensor_reduce(
                    out=s[:].rearrange("p (j pq) -> p j pq", pq=PS),
                    in_=ts[:].rearrange("p (j k pp q) -> p j pp q k",
                                        k=4, pp=4, q=4),
                    op=ALU.add, axis=AX.X)
                sv = s
            mz = work.tile([128, 128], F32, tag=f"mz{nl}", name="mz")
            n2 = small.tile([128, 8], F32, tag=f"n2{nl}", name="n2")
            meng = nc.gpsimd if (t == 2 and nl == 0) else nc.vector
            meng.tensor_tensor(
                mz[:], (mbar[:] if t == 1 else sv[:]), sv[:], op=ALU.mult)
            nc.vector.tensor_reduce(
                out=n2[:], in_=mz[:].rearrange("p (j kq) -> p j kq",
                                               kq=PS),
                op=ALU.add, axis=AX.X)
            n2c = small.tile([128, 8], F32, tag=f"n2c{nl}", name="n2c")
            nc.vector.tensor_scalar_max(n2c[:], n2[:], 0.0)
            # f = n2/(1+n2) * (n2+eps)^-0.5 via Ln/Exp (v1-proven path);
            # dd/rec/ff overlap the Act round-trip on DVE.
            tln = small.tile([128, 8], F32, tag=f"tln{nl}", name="tln")
            nc.scalar.activation(tln[:], n2c[:], AF.Ln, bias=epsc[:])
            dd = small.tile([128, 8], F32, tag=f"dd{nl}", name="dd")
            nc.vector.tensor_scalar_add(dd[:], n2c[:], 1.0)
            rec = small.tile([128, 8], F32, tag=f"rec{nl}", name="rec")
            nc.vector.reciprocal(rec[:], dd[:])
            rr = small.tile([128, 8], F32, tag=f"rr{nl}", name="rr")
            nc.scalar.activation(rr[:], tln[:], AF.Exp, scale=-0.5)
            ff = small.tile([128, 8], F32, tag=f"ff{nl}", name="ff")
            nc.vector.tensor_mul(ff[:], n2c[:], rec[:])
            ff2 = small.tile([128, 8], F32, tag=f"ff2{nl}", name="ff2")
            nc.vector.tensor_mul(ff2[:], ff[:], rr[:])
            return sv, ff2

        def emit_chain1(nl):
            """t=1 chain for one nl: recd/extract/squash/u16/ub."""
            mbps = mbps_t1[nl]
            recds = emit_recd(nl, mbps)
            mbar = state.tile([128, 128], F16, tag=f"mbar{nl}",
                              name="mbar")
            emit_extract(nl, mbps, recds, mbar)
            z, ff2 = emit_squash(nl, mbar, 1)
            fbc = ff2[:].unsqueeze(2).broadcast_to((128, 8, PS))
            u16 = state.tile([128, 128], F16, tag=f"u16{nl}", name="u16")
            nc.vector.tensor_tensor(
                u16[:].rearrange("p (j kq) -> p j kq", kq=PS),
                z[:].rearrange("p (j kq) -> p j kq", kq=PS),
                fbc, op=ALU.mult)
            ubt = state.tile([128, 128], F16, tag=f"ub{nl}", name="ubt")
            nc.vector.tensor_add(
                ubt[:], u0ub[:, nl * 128:(nl + 1) * 128], u16[:])
            return ubt

        utp_big = utps_pool.tile([128, 256], F16, tag="utp",
                                 name="utp_big")

        def emit_transp(nl, ubt):
            c0 = nl * 128
            nc.tensor.transpose(utp_big[:, c0:c0 + 128], ubt[:], ident)
            uta = state.tile([128, 128], F16, tag=f"uta{nl}", name="uta")
            nc.scalar.activation(uta[:], utp_big[:, c0:c0 + 128], AF.Copy)
            return uta

        def emit_chain2(nl):
            """t=2 chain for one nl: recd/extract/S/out + DMA."""
            mbps = mbps_t2[nl]
            recds = emit_recd(nl, mbps)
            mbar = state.tile([128, 128], F16, tag=f"mbar2{nl}",
                              name="mbar")
            emit_extract(nl, mbps, recds, mbar)
            s, ff2 = emit_squash(nl, mbar, 2)
            fbc = ff2[:].unsqueeze(2).broadcast_to((128, 8, PS))
            outsb = state.tile([128, 128], F32, tag=f"outsb{nl}",
                               name="outsb")
            oeng = nc.gpsimd if nl == 0 else nc.vector
            oeng.tensor_tensor(
                outsb[:].rearrange("p (j kq) -> p j kq", kq=PS),
                s[:].rearrange("p (j kq) -> p j kq", kq=PS),
                fbc, op=ALU.mult)
            src_o = outsb[:].rearrange("p (jj kq) -> p jj kq", kq=PS)
            dst_o = o32[nl].rearrange("(jj aL) b kq -> (aL b) jj kq", jj=J)
            deng = nc.gpsimd if nl == 0 else nc.sync
            deng.dma_start(out=dst_o, in_=src_o)

        # ================= pipelined schedule
        mbps_t1 = {}
        mbps_t2 = {}

        emit_L_mms(0, u0t32, 0)            # PE: L1-nl0
        mbps_t1[0] = emit_mb(0)            # PE: Mb1-nl0
        emit_L_mms(1, u0t32, 128)          # PE: L1-nl1  (chain1-nl0 overlaps)
        ub0 = emit_chain1(0)
        mbps_t1[1] = emit_mb(1)            # PE: Mb1-nl1
        uta0 = emit_transp(0, ub0)
        ub1 = emit_chain1(1)               # overlaps L2-nl0
        emit_L_mms(0, uta0, 0)             # PE: L2-nl0
        mbps_t2[0] = emit_mb(0)            # PE: Mb2-nl0
        uta1 = emit_transp(1, ub1)
        emit_chain2(0)                     # overlaps L2-nl1 on PE
        emit_L_mms(1, uta1, 0, jps=(0, 1))     # PE: L2-nl1 j0-3
        mbps_t2[1] = emit_mb(1, jhs=(0,))      # PE: Mb2-nl1 jh0
        emit_L_mms(1, uta1, 0, jps=(2, 3))     # PE: L2-nl1 j4-7
        emit_mb(1, jhs=(1,), mbps=mbps_t2[1])  # PE: Mb2-nl1 jh1
        emit_chain2(1)


def _build_kernel():
    nc = bacc.Bacc("TRN2", target_bir_lowering=False, debug=False,
                   num_devices=NCORES)
    mt32d = nc.dram_tensor("mt32", [128, 4096], F16,
                           kind="ExternalInput").ap()
    u0t32d = nc.dram_tensor("u0t32", [128, 256], F16,
                            kind="ExternalInput").ap()
    auxd = nc.dram_tensor("aux", [128, 1216], F32,
                          kind="ExternalInput").ap()
    xmbd = nc.dram_tensor("xmb", [128, 2080], BF16,
                          kind="ExternalInput").ap()
    o32 = nc.dram_tensor("o32", [NL, A, B, PS], F32,
                         kind="ExternalOutput").ap()

    with tile.TileContext(nc) as tc:
        _emit(tc, mt32d, u0t32d, auxd, xmbd, o32)

    nc.compile()
    return nc


# ---------------------------------------------------------------- host side
def _squashf(n2):
    n2c = np.maximum(n2, 0.0)
    return (n2c / (1.0 + n2c)) / np.sqrt(n2c + EPS)


def _host_prep(x, weights):
    xr = np.asarray(x, np.float32).reshape(BATCH, HW, A, PS)
    W = np.asarray(weights, np.float32)
    Gm = np.einsum("abpk,abpl->abkl", W, W)
    Gsw = np.swapaxes(Gm, 2, 3)                    # [a,b,kp,k] = Gm[..,k,kp]
    Wsw = np.swapaxes(W, 2, 3)                     # [a,b,k,pp] = W[..,pp,k]

    # wga[aL*32+b, j*64 + kp*16 + k*4 + q] = Gsw[4j+aL, b, kp, k]
    wga = np.empty((4, B, J, 4, 4, 4), np.float32)
    wws = np.empty((4, B, J, 4, 4, 4), np.float32)
    for j in range(J):
        wga[:, :, j] = Gsw[4 * j:4 * j + 4, :, :, :, None]
        wws[:, :, j] = Wsw[4 * j:4 * j + 4, :, :, :, None]
    wga = np.ascontiguousarray(wga.reshape(128, 512))
    wws = np.ascontiguousarray(wws.reshape(128, 512))
    ident = np.eye(128, dtype=np.float16)

    x16 = xr.astype(np.float16)                    # L-path
    xbf = xr.astype(ml_dtypes.bfloat16)            # Mb-path

    in_maps = []
    for c in range(NCORES):
        xc16 = x16[c * NL:(c + 1) * NL]            # (2, 256, 32, 16)
        xcbf = xbf[c * NL:(c + 1) * NL]
        xcf = xr[c * NL:(c + 1) * NL]

        # host U0 (t=0 state; softmax at t=0 is uniform)
        Mbar0 = xcf.mean(axis=1)                   # (2, A, PS)
        Z0 = np.einsum("abkl,nalq->nabkq", Gm,
                       Mbar0.reshape(NL, A, P, P)).reshape(NL, A, B, PS)
        n2_0 = np.einsum("nak,nabk->nab", Mbar0, Z0)[..., None]
        U0 = (_squashf(n2_0) * Z0).astype(np.float16)   # (2, A, B, PS)

        # u0t32[j*16+kq, nl*128 + aL*32 + b] ; u0ub[aL*32+b, nl*128+(j,kq)]
        u0t32 = np.zeros((128, 256), np.float16)
        u0ub = np.zeros((128, 256), np.float16)
        for nl in range(NL):
            for j in range(J):
                for aL in range(4):
                    blk = U0[nl, 4 * j + aL]       # (B, PS)
                    u0t32[j * 16:(j + 1) * 16,
                          nl * 128 + aL * 32:nl * 128 + (aL + 1) * 32] = \
                        blk.T
                    u0ub[aL * 32:(aL + 1) * 32,
                         nl * 128 + j * 16:nl * 128 + (j + 1) * 16] = blk

        # mt32[j2*32 + par*16 + kq, ((nl*2+par)*4+aL)*256 + ch*128 + h]
        mt32 = np.zeros((128, 4096), np.float16)
        for j in range(J):
            j2, par = j // 2, j % 2
            for nl in range(NL):
                for aL in range(4):
                    for ch in range(2):
                        col = ((nl * 2 + par) * 4 + aL) * 256 + ch * 128
                        mt32[j2 * 32 + par * 16:j2 * 32 + par * 16 + 16,
                             col:col + 128] = \
                            xc16[nl, ch * 128:(ch + 1) * 128,
                                 4 * j + aL, :].T

        # xmb[h, nl*1040 + ch*520 + j*65 + (aL*16+kq | 64)]
        xmb = np.empty((128, 2080), ml_dtypes.bfloat16)
        xv = xmb.reshape(128, NL, 2, J, 65)
        for nl in range(NL):
            for ch in range(2):
                xv[:, nl, ch, :, :64] = (
                    xcbf[nl, ch * 128:(ch + 1) * 128]
                    .reshape(128, J, 64))
                xv[:, nl, ch, :, 64] = 1.0

        aux = np.concatenate([
            ident.view(np.float32), wga, u0ub.view(np.float32), wws,
        ], axis=1)
        in_maps.append({
            "mt32": mt32,
            "u0t32": u0t32,
            "aux": np.ascontiguousarray(aux),
            "xmb": np.ascontiguousarray(xmb),
        })
    return in_maps


_NC_CACHE = {}


def kernel(x, weights):
    if "nc" not in _NC_CACHE:
        _NC_CACHE["nc"] = _build_kernel()
    nc = _NC_CACHE["nc"]
    in_maps = _host_prep(x, weights)
    res = run_bass_kernel_spmd(nc, in_maps, list(range(NCORES)))
    out = np.concatenate([res.results[c]["o32"] for c in range(NCORES)],
                         axis=0)
    return out.astype(np.float32)
